# revision 70
# baseline (speedup 1.0000x reference)
"""Causal MHA (B=4, T=2048, D=1024, H=16) on 8 trn2 cores.

Sharding: core c = (batch b = c//2, head-group g = c%2). Each core computes
QKV projections for its 8 heads, causal attention, and the row-parallel
out-proj partial product. Host sums the two partials per batch + bias.

On-device layout (per core):
  xT   [1024, 2048]  X^T (d on partitions)           bf16
  QT/KT [512, 2048]  Q^T/K^T (e=head*64+d rows)      bf16
  V_pad [2048, 520]  V natural + ones col per head   bf16
  scores S^T tiles [128 k, 2x512 q] (2 heads/psum, concurrent row-tiled
  matmuls), exp on ScalarE, causal handled at column granularity: for
  q-block j and k-chunk i=4j+r the first 128r columns are fully masked,
  so S matmul / exp / select all skip them; only the 128-wide diagonal
  block needs an affine_select.
  ctx = P^T-stationary matmul -> [128 q, 65] (col 64 = softmax denom),
  normalize per-partition, transpose -> ctx^T (DMA xbar for cp<3; PE
  transpose for the last head-pair, whose transposes sit on the
  out-proj critical chain), out-proj partial.
  Output DMA'd as fp16 (host sums the two partials in fp32 + bias).
  Dummy identity matmuls during the input-DMA wait open the PE HAM
  clock gate so real projections start at full clock.
"""

import os

import numpy as np
import ml_dtypes

import concourse.bass as bass
import concourse.bacc as bacc
import concourse.tile as tile
from concourse import mybir
from concourse.bass_utils import run_bass_kernel_spmd
from concourse.masks import make_identity

BF16 = ml_dtypes.bfloat16

B, T, D = 4, 2048, 1024
H, HD = 16, 64
E = 512          # per-core projection width (8 heads * 64)
DC = D // 128    # 8 contraction chunks
EC = E // 128    # 4 e chunks (head pairs)
TJ = T // 512    # 4 q-chunks of 512
TQ = T // 128    # 16 t-chunks of 128

F32 = mybir.dt.float32
F16 = mybir.dt.float16
BF = mybir.dt.bfloat16

LAST = {}
_CACHE = {}


def _build():
    nc = bacc.Bacc("TRN2")
    xTp = nc.dram_tensor("xTp", [4, DC, 128, 512], BF, kind="ExternalInput")
    wq = nc.dram_tensor("wq", [D, E], BF, kind="ExternalInput")
    wk = nc.dram_tensor("wk", [D, E], BF, kind="ExternalInput")
    wv = nc.dram_tensor("wv", [D, E], BF, kind="ExternalInput")
    wo = nc.dram_tensor("wo", [E, D], BF, kind="ExternalInput")
    outp = nc.dram_tensor("out", [T, D], F16, kind="ExternalOutput")

    with tile.TileContext(nc) as tc:
        with (
            tc.tile_pool(name="const", bufs=1) as const,
            tc.tile_pool(name="acts", bufs=1) as acts,
            tc.tile_pool(name="ppool", bufs=26) as ppool,
            tc.tile_pool(name="small", bufs=6) as small,
            tc.tile_pool(name="stage", bufs=6) as stage,
            tc.tile_pool(name="obuf", bufs=2) as obufp,
            tc.tile_pool(name="psS", bufs=2, space="PSUM") as psS,
            tc.tile_pool(name="psP", bufs=2, space="PSUM") as psP,
            tc.tile_pool(name="psC", bufs=2, space="PSUM") as psC,
        ):
            wq_sb = const.tile([128, DC, E], BF, tag="wq")
            wk_sb = const.tile([128, DC, E], BF, tag="wk")
            wv_sb = const.tile([128, DC, E], BF, tag="wv")
            wo_sb = const.tile([128, EC, D], BF, tag="wo")
            ident = const.tile([128, 128], BF, tag="ident")
            ident32 = const.tile([128, 128], F32, tag="ident32")
            xT_sb = acts.tile([128, DC, T], BF, tag="xT")

            def dma_x_slice(tcc):
                nc.sync.dma_start(
                    out=xT_sb[:, :, tcc * 512 : (tcc + 1) * 512],
                    in_=xTp[tcc].rearrange("dc p f -> p dc f"),
                )

            nc.sync.dma_start(out=wq_sb, in_=wq.rearrange("(dc p) e -> p dc e", p=128))
            dma_x_slice(0)
            nc.sync.dma_start(out=wk_sb, in_=wk.rearrange("(dc p) e -> p dc e", p=128))
            dma_x_slice(1)
            nc.sync.dma_start(
                out=wv_sb, in_=wv.rearrange("(dc p) e -> p dc e", p=128)
            )
            dma_x_slice(2)
            dma_x_slice(3)
            nc.sync.dma_start(
                out=wo_sb, in_=wo.rearrange("(ec p) o -> p ec o", p=128)
            )
            make_identity(nc, ident)
            make_identity(nc, ident32)

            QT_sb = acts.tile([128, EC, T], BF, tag="QT")
            KT_sb = acts.tile([128, EC, T], BF, tag="KT")
            V_sb = acts.tile([128, TQ, 8 * 65], BF, tag="V")
            CT_sb = acts.tile([128, EC, T], BF, tag="CT")

            # ones columns only (col 64 of each per-head 65-group) so the V
            # copies below touch disjoint bytes and carry no WAW dep here
            for t7 in range(TQ):
                nc.vector.memset(
                    V_sb[:, t7, :].rearrange("p (h d) -> p h d", d=65)[:, :, 64:65],
                    1.0,
                )

            # ---------- op factories (emitted interleaved, see queue below)
            def qk_op(di, cp, t5):
                dst, w_sb = ((QT_sb, wq_sb), (KT_sb, wk_sb))[di]

                def op(dst=dst, w_sb=w_sb, cp=cp, t5=t5):
                    ps = psP.tile([128, 512], F32, tag="psP")
                    for dc in range(DC):
                        nc.tensor.matmul(
                            ps,
                            lhsT=w_sb[:, dc, cp * 128 : (cp + 1) * 128],
                            rhs=xT_sb[:, dc, t5 * 512 : (t5 + 1) * 512],
                            start=(dc == 0),
                            stop=(dc == DC - 1),
                        )
                    nc.vector.tensor_copy(
                        out=dst[:, cp, t5 * 512 : (t5 + 1) * 512], in_=ps
                    )

                return op

            def v_op(t7):
                def op(t7=t7):
                    ps = psP.tile([128, 512], F32, tag="psP")
                    for dc in range(DC):
                        nc.tensor.matmul(
                            ps,
                            lhsT=xT_sb[:, dc, t7 * 128 : (t7 + 1) * 128],
                            rhs=wv_sb[:, dc, :],
                            start=(dc == 0),
                            stop=(dc == DC - 1),
                        )
                    nc.vector.tensor_copy(
                        out=V_sb[:, t7, :].rearrange(
                            "p (h d) -> p h d", d=65
                        )[:, :, 0:64],
                        in_=ps.rearrange("p (h d) -> p h d", d=64),
                    )

                return op

            def outproj_op(t7):
                def op(t7=t7):
                    ob = obufp.tile([128, 1024], F16, tag="obuf")
                    pss = []
                    for oc in range(2):
                        ps = psP.tile([128, 512], F32, tag="psP")
                        for ec in range(EC):
                            nc.tensor.matmul(
                                ps,
                                lhsT=CT_sb[:, ec, t7 * 128 : (t7 + 1) * 128],
                                rhs=wo_sb[:, ec, oc * 512 : (oc + 1) * 512],
                                start=(ec == 0),
                                stop=(ec == EC - 1),
                            )
                        pss.append(ps)
                    # casts after BOTH matmul chains: when the DVE reaches
                    # cast oc=0, its matmuls finished ~850ns ago, so the
                    # DVE FIFO no longer head-of-line blocks on the PE
                    for oc in range(2):
                        nc.vector.tensor_copy(
                            out=ob[:, oc * 512 : (oc + 1) * 512], in_=pss[oc]
                        )
                    nc.sync.dma_start(
                        out=outp[t7 * 128 : (t7 + 1) * 128, :], in_=ob
                    )

                return op

            def ctx_qr_ops(cp, j, qr, pts):
                qc = 4 * j + qr
                last_cp = cp == EC - 1
                if last_cp:
                    cn = stage.tile([128, 128], F32, tag="ctxn32")
                else:
                    cn = stage.tile([128, 128], BF, tag="ctxn")
                cell = {}
                ops = []
                for h in range(2):

                    def mm_group(cp=cp, h=h, qr=qr, qc=qc, pts=pts, cn=cn,
                                 last_cp=last_cp, cell=cell):
                        habs = 2 * cp + h
                        if last_cp:
                            # short per-group chain: normalize immediately
                            cpst = psC.tile([128, 130], F32, tag="psC")
                            cps = cpst[:, 0:65]
                        else:
                            # share one bank per qr pair; DVE after both
                            # chains so the PE never writes a bank the DVE
                            # is reading
                            if h == 0:
                                cpair = psC.tile([128, 130], F32, tag="psC")
                                cell["t"] = cpair
                            cps = cell["t"][:, 65 * h : 65 * h + 65]
                        for i in range(qc + 1):
                            nc.tensor.matmul(
                                cps,
                                lhsT=pts[i][:, h, qr * 128 : (qr + 1) * 128],
                                rhs=V_sb[:, i, habs * 65 : habs * 65 + 65],
                                start=(i == 0),
                                stop=(i == qc),
                            )
                        if last_cp:
                            rc = small.tile([128, 1], F32, tag="recip")
                            nc.vector.reciprocal(rc, cps[:, 64:65])
                            nc.vector.tensor_scalar_mul(
                                out=cn[:, 64 * h : 64 * h + 64],
                                in0=cps[:, 0:64],
                                scalar1=rc,
                            )
                        elif h == 1:
                            # one strided reciprocal covers both heads'
                            # denominators (cols 64 and 129)
                            rc2 = small.tile([128, 2], F32, tag="recip2")
                            nc.vector.reciprocal(
                                rc2,
                                cell["t"].rearrange(
                                    "p (hh d) -> p hh d", d=65
                                )[:, :, 64],
                            )
                            for hh in range(2):
                                nc.vector.tensor_scalar_mul(
                                    out=cn[:, 64 * hh : 64 * hh + 64],
                                    in0=cell["t"][:, 65 * hh : 65 * hh + 64],
                                    scalar1=rc2[:, hh : hh + 1],
                                )

                    ops.append(mm_group)

                def finish(cp=cp, qc=qc, cn=cn):
                    if cp == EC - 1:
                        # last head-pair: the transpose sits on the
                        # ctx->out-proj critical chain; PE transpose +
                        # DVE copy (~0.6us) beats the ~1.2us+sem DMA
                        # xbar path, and the PE idles here anyway
                        tps = psC.tile([128, 130], F32, tag="psC")
                        nc.tensor.transpose(tps[:, 0:128], cn, ident32)
                        nc.vector.tensor_copy(
                            out=CT_sb[:, cp, qc * 128 : (qc + 1) * 128],
                            in_=tps[:, 0:128],
                        )
                    else:
                        nc.sync.dma_start_transpose(
                            out=CT_sb[:, cp, qc * 128 : (qc + 1) * 128],
                            in_=cn,
                        )

                ops.append(finish)
                return ops

            def ctx_ops(cp, j, pts):
                ops = []
                for qr in range(4):
                    ops.extend(ctx_qr_ops(cp, j, qr, pts))
                return ops

            # ---------- pipelined emission ----------
            # S/exp/mask stream is ACT-bound; all other PE work (projections,
            # V, ctx of the previous iteration, out-proj) drains through the
            # two queues between S steps so the PE never starves.
            # prepay first-call costs during the input-DMA wait: the exp
            # ACT_TABLE_LOAD (~2.7us) and GpSimd dispatch warm-up would
            # otherwise land on the first S tile's critical chain
            scr = stage.tile([128, 16], BF, tag="warm")
            scr2 = stage.tile([128, 16], BF, tag="warm2")
            nc.vector.memset(scr, 0.0)
            nc.scalar.activation(
                out=scr2, in_=scr,
                func=mybir.ActivationFunctionType.Exp, scale=0.125,
            )
            nc.gpsimd.affine_select(
                out=scr2, in_=scr2,
                compare_op=mybir.AluOpType.is_ge, fill=0.0,
                base=0, pattern=[[1, 16]], channel_multiplier=-1,
            )
            # HAM warmup: the PE clock-gate opens only after ~3.4us of
            # sustained matmul activity; burn dummy identity matmuls during
            # the otherwise-idle input-DMA wait so the real projections run
            # at 2.4GHz from the start
            for _ in range(80):
                wps = psP.tile([128, 512], F32, tag="psP")
                nc.tensor.matmul(
                    wps[:, 0:128], lhsT=ident, rhs=ident, start=True, stop=True
                )

            first_order = [
                (0, 0), (1, 0), (0, 1), (1, 1),
                (0, 2), (1, 2), (0, 3), (1, 3),
            ]
            for di, t5 in first_order:
                qk_op(di, 0, t5)()
            slow = [v_op(t7) for t7 in range(TQ)]
            fast = []
            for cp in range(EC):
                if cp < EC - 1:
                    slow.extend(
                        qk_op(di, cp + 1, t5) for t5 in range(TJ) for di in range(2)
                    )
                steps_left = 40
                for j in range(TJ):
                    nk = 4 * j + 4
                    if cp == EC - 1 and j == TJ - 1:
                        ds = nk  # spread filler across the whole last phase
                    else:
                        ds = max(1, nk // 2)
                    fper = (len(fast) + ds - 1) // ds
                    pts = []
                    for i in range(nk):
                        r = i - 4 * j  # >= 0 on staircase/diagonal tiles
                        c0 = 128 * r if r > 0 else 0  # cols < c0 fully masked
                        pt = ppool.tile([128, 2, 512], BF, tag="P")
                        sh = psS.tile([128, 2, 512], F32, tag="psS")
                        for h in range(2):
                            lo = 64 * h
                            nc.tensor.matmul(
                                sh[:, h, c0:512],
                                lhsT=KT_sb[lo : lo + 64, cp, i * 128 : (i + 1) * 128],
                                rhs=QT_sb[lo : lo + 64, cp, j * 512 + c0 : (j + 1) * 512],
                                start=True,
                                stop=True,
                            )
                        nc.scalar.activation(
                            out=pt[:, :, c0:512],
                            in_=sh[:, :, c0:512],
                            func=mybir.ActivationFunctionType.Exp,
                            scale=0.125,
                        )
                        if r >= 0:
                            # only the 128-wide diagonal block needs the
                            # causal select (left of it: skipped entirely;
                            # right of it: fully valid)
                            for h in range(2):
                                nc.gpsimd.affine_select(
                                    out=pt[:, h, 128 * r : 128 * r + 128],
                                    in_=pt[:, h, 128 * r : 128 * r + 128],
                                    compare_op=mybir.AluOpType.is_ge,
                                    fill=0.0,
                                    base=0,
                                    pattern=[[1, 128]],
                                    channel_multiplier=-1,
                                )
                        pts.append(pt)
                        for _ in range(fper):
                            if fast:
                                fast.pop(0)()
                        if cp == EC - 1 and j == TJ - 1 and r >= 0:
                            # final phase: each qr chain depends only on its
                            # own diagonal tile; with the short PE-transpose
                            # chain, inline emission leaves only the qr=3
                            # chain as the kernel tail
                            for fop in ctx_qr_ops(cp, j, r, pts):
                                fop()
                            outproj_op(4 * j + r)()
                            continue
                        spr = (len(slow) + steps_left - 1) // steps_left
                        for _ in range(spr):
                            if slow:
                                slow.pop(0)()
                        steps_left -= 1
                    while fast:
                        fast.pop(0)()
                    if cp == EC - 1 and j == TJ - 1:
                        fast = []
                        continue
                    fast = ctx_ops(cp, j, pts)
                    if cp == EC - 1:
                        # out-proj for these q rows AFTER their ctx writes in
                        # the same FIFO (queued ops can only depend on already
                        # emitted producers). Mid-kernel phases interleave
                        # per qr; the final phase (drained with no S steps
                        # left to hide transpose latency) runs all ctx
                        # chains first so the transposes complete behind
                        # ctx PE work, then the out-projs back-to-back.
                        if j < TJ - 1:
                            mix = []
                            for qr in range(4):
                                mix.extend(fast[qr * 3 : qr * 3 + 3])
                                mix.append(outproj_op(4 * j + qr))
                        else:
                            mix = list(fast)
                            mix.extend(outproj_op(4 * j + qr) for qr in range(4))
                        fast = mix
            while fast:
                fast.pop(0)()
            while slow:
                slow.pop(0)()
    nc.compile()
    return nc


def _get_nc():
    if "nc" not in _CACHE:
        _CACHE["nc"] = _build()
    return _CACHE["nc"]


def _ensure_ntff_hook():
    """Install the axon NTFF profiling hook if the image's antenv lacks it."""
    import sys
    import types

    try:
        import antenv.axon_hooks  # noqa: F401

        return
    except ImportError:
        pass
    try:
        import antenv

        mod = types.ModuleType("antenv.axon_hooks")
        holder = {"hook": None}
        mod.set_axon_ntff_profile_hook = lambda h: holder.__setitem__("hook", h)
        mod.get_axon_ntff_profile_hook = lambda: holder["hook"]
        sys.modules["antenv.axon_hooks"] = mod
        antenv.axon_hooks = mod
        from trn_agent_boot.trn_boot import _ntff_profile_via_ctypes

        so = "/opt/axon/libaxon_pjrt.so"
        if os.path.exists(so):
            mod.set_axon_ntff_profile_hook(_ntff_profile_via_ctypes(so))
    except Exception:
        pass


def kernel(inputs, Wq, Wk, Wv, Wo, bo):
    inputs = np.asarray(inputs, dtype=np.float32)
    Wq = np.asarray(Wq, dtype=np.float32)
    Wk = np.asarray(Wk, dtype=np.float32)
    Wv = np.asarray(Wv, dtype=np.float32)
    Wo = np.asarray(Wo, dtype=np.float32)
    bo = np.asarray(bo, dtype=np.float32)

    nc = _get_nc()
    wqs = [np.ascontiguousarray(Wq[:, g * E : (g + 1) * E]).astype(BF16) for g in range(2)]
    wks = [np.ascontiguousarray(Wk[:, g * E : (g + 1) * E]).astype(BF16) for g in range(2)]
    wvs = [np.ascontiguousarray(Wv[:, g * E : (g + 1) * E]).astype(BF16) for g in range(2)]
    wos = [np.ascontiguousarray(Wo[g * E : (g + 1) * E, :]).astype(BF16) for g in range(2)]
    xTs = [
        np.ascontiguousarray(
            inputs[b].T.reshape(DC, 128, 4, 512).transpose(2, 0, 1, 3)
        ).astype(BF16)
        for b in range(B)
    ]

    in_maps = []
    for c in range(8):
        b, g = divmod(c, 2)
        in_maps.append(
            {
                "xTp": xTs[b],
                "wq": wqs[g],
                "wk": wks[g],
                "wv": wvs[g],
                "wo": wos[g],
            }
        )

    trace = os.environ.get("KERNEL_TRACE", "0") == "1"
    if trace:
        _ensure_ntff_hook()
    tcores = None
    if os.environ.get("KERNEL_TRACE_ALL", "0") == "1":
        tcores = list(range(8))
    res = run_bass_kernel_spmd(
        nc, in_maps, core_ids=list(range(8)), trace=trace, trace_cores=tcores
    )
    LAST["exec_ns"] = res.exec_time_ns
    LAST["trace"] = res.instructions_and_trace
    LAST["profile_json"] = res.profile_json

    out = np.empty((B, T, D), np.float32)
    for b in range(B):
        out[b] = (
            res.results[2 * b]["out"].astype(np.float32)
            + res.results[2 * b + 1]["out"].astype(np.float32)
            + bo[None, :]
        )
    return out
